# revision 49
# baseline (speedup 1.0000x reference)
"""Trainium2 Bass kernel for nn_ByteSequenceEmbedder — fp8 DoubleRow version.

Per core (1 sequence, 8 sequences data-parallel over 8 cores):
  x  = tok_emb[tokens] + bpe*E[4] + word*E[3]                 [T=4096, 64]
  x  = relu(conv3(x, W0) + b0); 2x highway(512)               [T, 512]
  x  = relu(conv3(x, W1) + b1 + x); 2x highway(512)           [T, 512]
  x  = per-word segment max (ragged, sorted seg_ids, W=1024)  [W, 512]
  out= x @ Pw + Pb                                            [W, 512]

Numerics (validated in numsim.py, rel err ~1.69e-2):
 - masters bf16, scaled by A=64, nonnegative
 - hw g-matmuls: plain fp8e4 DoubleRow (K=16-scaled weights)
 - hw h-matmuls: full hi/lo fp8 split (yq@Wh + yl@Wh + yq@Wl)
 - conv1: weight-split + bf16 identity tap (K*I) for the residual
 - conv0 / proj / transpose / segmax: bf16

Scheduling: evictions/combines at 1024-token pair granularity (2-bank
PSUM tiles); the embedding gathers go ahead of the (chunked) weight
DMAs; PE p-state is pre-warmed with dummy transposes and the act table
preloaded with a dummy sigmoid; each phase emits the next phase's
fp8 converts per-chunk so they pipeline instead of serializing at
phase boundaries; segmax gathers use tight per-chunk row bounds.
"""

import functools
import os
import sys

import numpy as np

for _p in ("/opt/trn_rl_repo", "/root/.axon_site/_ro/trn_rl_repo"):
    if os.path.isdir(_p) and _p not in sys.path:
        sys.path.append(_p)

import ml_dtypes  # noqa: E402

from contextlib import ExitStack  # noqa: E402

from concourse import bacc, bass, mybir, tile  # noqa: E402
from concourse import library_config  # noqa: E402
from concourse.bass_utils import run_bass_kernel_spmd  # noqa: E402
from concourse.dve_ops import AFFINE_THEN_ADD  # noqa: E402

B, T, W = 8, 4096, 1024
DB, DW = 64, 512
NH = 2
VOCAB = 264
BPE_MARK, WORD_MARK = 4, 3
PSC = 512          # tokens per psum bank
PAIR = 1024        # tokens per paired psum tile (2 banks)
NPR = T // PAIR    # 4 pairs
YQS = 1024         # tokens per yq/yl chunk == PAIR
MCH = DW // 128
KCH = DW // 128
NCORES = 8
CVOCAB = 4 * VOCAB
A_SCALE = 64.0
K_SCALE = 16.0
GCH = (1024, 2048, 3072, 4096)   # gather chunk boundaries

BF16 = mybir.dt.bfloat16
F32 = mybir.dt.float32
FP8 = mybir.dt.float8e4
I16 = mybir.dt.int16
AF = mybir.ActivationFunctionType
OP = mybir.AluOpType
PM = mybir.MatmulPerfMode

bf16_np = ml_dtypes.bfloat16
fp8_np = ml_dtypes.float8_e4m3


def build_program(ntaps: int, rmaxs: tuple, stage: int = 9,
                  skip_lo: tuple = (0, 1), warm: int = 40,
                  ps_bufs: int = 3, ptp_bufs: int = 1,
                  fold_seg: int = 0, fold_proj: bool = False,
                  tail_warm: int = 0, y1t_split: bool = False,
                  proj_alt: bool = False, cvt_sync: bool = False,
                  stack0: bool = True, last_dve: bool = False,
                  gp_bufs: int = 3, wkp_bufs: int = 3,
                  tp_bufs: int = 3, c1p_bufs: int = 4, ylp_bufs: int = 4,
                  phase_offsets: tuple = (0, 2, 4, 7, 9, 11),
                  tree_pool: bool = False, gat_bufs: int = 3,
                  cvt_halves: bool = False, yl_tt: bool = True,
                  yl_pool: bool = True, w1_half: bool = True,
                  proj_pair: bool = False,
                  yqp_bufs: int = 4, mst_bufs: int = 8,
                  routing: dict | None = None) -> bass.Bass:
    FOLD_SEG, FOLD_PROJ = int(fold_seg), fold_proj
    TAIL_WARM = tail_warm
    Y1T_SPLIT = y1t_split
    PROJ_ALT = proj_alt
    CVT_SYNC = cvt_sync
    STACK0 = stack0
    LAST_DVE = last_dve
    TREE_POOL = tree_pool
    CVT_HALVES = cvt_halves
    YL_TT = yl_tt
    YL_POOL = yl_pool
    W1_HALF = w1_half
    PROJ_PAIR = proj_pair
    nc = bacc.Bacc("TRN2", target_bir_lowering=False, debug=False)

    def din(name, shape, dtype):
        return nc.dram_tensor(name, list(shape), dtype, kind="ExternalInput")

    emb_d = din("emb_comb", (CVOCAB, 128), BF16)
    tokidx_d = din("tok_idx", (128, T // 16), I16)
    w0_d = din("w0", (DB, 3, DW), BF16)
    w0s_d = din("w0s", (128, DW), BF16)
    wg_d = din("wg8", (128, 4, KCH, DW), FP8)
    whh_d = din("wh8", (128, 4, KCH, DW), FP8)
    whl_d = din("wl8", (128, 4, KCH, DW), FP8)
    w1h_d = din("w1h8", (128, 3, KCH, DW), FP8)
    w1l_d = din("w1l8", (128, 3, KCH, DW), FP8)
    id16_d = din("id16", (128, 128), BF16)
    ident_d = din("ident", (128, 128), BF16)
    projw_d = din("projw", (128, KCH, DW), BF16)
    projb_d = din("projb", (1, DW), BF16)
    bias_d = din("biases", (128, 24), F32)
    gidx_d = din("gidx", (128, 8 * ntaps * 8), I16)
    out_d = nc.dram_tensor("out", [W, DW], F32, kind="ExternalOutput")
    y1t_d = nc.dram_tensor("y1t", [T, DW], BF16, kind="Internal")

    inv_ak = 1.0 / (A_SCALE * K_SCALE)
    inv_k = 1.0 / K_SCALE

    with tile.TileContext(nc) as tc, ExitStack() as ctx:
        const = ctx.enter_context(tc.tile_pool(name="const", bufs=1))
        xap = ctx.enter_context(tc.tile_pool(name="xap", bufs=1))
        ps = ctx.enter_context(tc.tile_pool(name="psp", bufs=ps_bufs,
                                            space="PSUM"))
        ptp = (ctx.enter_context(tc.tile_pool(name="ptp", bufs=ptp_bufs,
                                              space="PSUM"))
               if ptp_bufs else ps)
        ppp = ctx.enter_context(tc.tile_pool(name="ppp", bufs=1,
                                             space="PSUM"))

        mst = ctx.enter_context(tc.tile_pool(name="mst", bufs=mst_bufs))
        yqp = ctx.enter_context(tc.tile_pool(name="yqp", bufs=yqp_bufs))
        c1p = ctx.enter_context(tc.tile_pool(name="c1p", bufs=c1p_bufs))
        ylp = ctx.enter_context(tc.tile_pool(name="ylp", bufs=ylp_bufs))
        gp = ctx.enter_context(tc.tile_pool(name="gpool", bufs=gp_bufs))
        wkp = ctx.enter_context(tc.tile_pool(name="wkp", bufs=wkp_bufs))
        tp = ctx.enter_context(tc.tile_pool(name="tpool", bufs=tp_bufs))
        gat = ctx.enter_context(tc.tile_pool(name="gat", bufs=gat_bufs))
        obp = ctx.enter_context(tc.tile_pool(name="obp", bufs=2))

        nc.gpsimd.load_library(library_config.mlp)

        def load(dram_t, shape, dtype, name, src=None):
            t = const.tile(shape, dtype, name=name)
            nc.sync.dma_start(out=t[:], in_=src if src is not None
                              else dram_t[:])
            return t

        # small consts on the SP queue, in consumption order
        tokidx_sb = load(tokidx_d, [128, T // 16], I16, "tokidx_sb")
        ident_sb = load(ident_d, [128, 128], BF16, "ident_sb")
        w0_sb = load(w0_d, [DB, 3, DW], BF16, "w0_sb")
        w0s_sb = load(w0s_d, [128, DW], BF16, "w0s_sb")
        bias_sb = load(bias_d, [128, 24], F32, "bias_sb")

        # preload the activation table set that serves sigmoid+relu+copy
        sigscr = const.tile([1, 8], F32, name="sigscr")
        nc.scalar.activation(out=sigscr[:], in_=bias_sb[0:1, 0:8],
                             func=AF.Sigmoid)

        # PE warmup: dummy transposes to ramp the p-state before conv0
        if warm:
            wps = ptp.tile([128, PAIR], BF16,
                           tag="pt" if ptp_bufs else "ps", name="warm")
            for i in range(warm):
                nc.tensor.transpose(
                    out=wps[:, 0:128],
                    in_=ident_sb[:], identity=ident_sb[:])

        # ---- embedding gather: ahead of the big weight DMAs ----
        xg = xap.tile([128, T], BF16, tag="xa", name="xg")
        if stage >= 1:
            prev = 0
            for r, hi in enumerate(GCH):
                ec = hi - prev
                nc.gpsimd.dma_gather(
                    out_ap=xg[:, prev:hi].rearrange(
                        "p (c n) -> p c n", c=1),
                    in_ap=emb_d[:],
                    idxs_ap=tokidx_sb[:, prev // 16:hi // 16],
                    num_idxs=ec,
                    num_idxs_reg=ec,
                    elem_size=128,
                    transpose=True,
                    single_packet=False,
                )
                if STACK0:
                    # xg[64:128, t] = xg[0:64, t+1]; col T-1 keeps zeros
                    lo = max(prev - 1, 0)
                    nc.sync.dma_start(
                        out=xg[64:128, lo:hi - 1],
                        in_=xg[0:DB, lo + 1:hi])
                prev = hi
        else:
            nc.vector.memset(xg[:], 0.0)

        # big weights: chunked DMAs on the SP queue so individual
        # transfers interleave with (rather than blocking) the gathers
        def loadc(dram_t, shape, dtype, name, nch):
            t = const.tile(shape, dtype, name=name)
            for c in range(nch):
                sl = slice(c * shape[1] // nch, (c + 1) * shape[1] // nch)
                nc.sync.dma_start(out=t[:, sl], in_=dram_t[:, sl])
            return t

        wg_sb = loadc(wg_d, [128, 4, KCH, DW], FP8, "wg_sb", 4)
        whh_sb = loadc(whh_d, [128, 4, KCH, DW], FP8, "whh_sb", 4)
        whl_sb = loadc(whl_d, [128, 4, KCH, DW], FP8, "whl_sb", 4)
        w1h_sb = loadc(w1h_d, [128, 3, KCH, DW], FP8, "w1h_sb", 3)
        w1l_sb = loadc(w1l_d, [128, 3, KCH, DW], FP8, "w1l_sb", 3)
        id16_sb = load(id16_d, [128, 128], BF16, "id16_sb")
        projw_sb = loadc(projw_d, [128, KCH, DW], BF16, "projw_sb", 2)
        projb_sb = load(projb_d, [1, DW], BF16, "projb_sb")
        gidx_sb = load(gidx_d, [128, 8 * ntaps * 8], I16, "gidx_sb")
        ones_sb = const.tile([1, 128], BF16, name="ones_sb")
        nc.vector.memset(ones_sb[:], 1.0)

        # routing[phase] = (act_num, act_den, add_swdge_num, add_den):
        # fraction act_num/act_den of relu evictions go to ACT (rest DVE);
        # fraction add_swdge_num/add_den of combine adds go to SWDGE.
        route = {"conv0": (2, 4, 0, 1), "hw0": (2, 3, 4, 4),
              "hw1": (3, 4, 3, 4), "conv1": (3, 4, 0, 1),
              "hwy": (3, 4, 3, 4), "tr": (1, 4, 0, 1), "proj": (4, 4, 0, 1)}
        if routing:
            route.update(routing)
        ev_ct = {}

        def evict_relu(dst, pc, m, scale_mode, phase):
            """dst = relu(pc*scale + bias) routed ACT/DVE per phase."""
            n = ev_ct.get(phase, 0)
            ev_ct[phase] = n + 1
            an, ad = route[phase][0], route[phase][1]
            use_act = (n % ad) < an
            if scale_mode == "conv0":
                if use_act:
                    nc.scalar.activation(
                        out=dst, in_=pc, func=AF.Relu,
                        bias=bias_sb[:, m:m + 1])
                else:
                    nc.vector.tensor_scalar(
                        out=dst, in0=pc,
                        scalar1=bias_sb[:, m:m + 1], scalar2=0.0,
                        op0=OP.add, op1=OP.max)
            else:  # inv_k scaled relu, zero bias
                if use_act:
                    nc.scalar.activation(
                        out=dst, in_=pc, func=AF.Relu, scale=inv_k,
                        bias=bias_sb[:, 20 + m:21 + m])
                else:
                    nc.vector.tensor_scalar(
                        out=dst, in0=pc, scalar1=0.0, scalar2=inv_k,
                        op0=OP.max, op1=OP.mult)

        def emit_cvt(src_ch, want_lo, lname, pool=None):
            yq = (pool or yqp).tile([128, KCH, YQS], FP8, tag="yq",
                                    name=f"yq{lname}")
            cq = nc.sync if CVT_SYNC else nc.gpsimd
            if CVT_HALVES:
                for hh in range(2):
                    cq.dma_start(
                        out=yq[:, :, hh * PSC:(hh + 1) * PSC],
                        in_=src_ch[:, :, hh * PSC:(hh + 1) * PSC])
            else:
                cq.dma_start(out=yq[:], in_=src_ch[:, :, :])
            yl = None
            if want_lo:
                yl = ylp.tile([128, KCH, YQS], FP8, tag="yl",
                              name=f"yl{lname}")
                if YL_TT:
                    for hh in range(2):
                        eng = nc.gpsimd if (YL_POOL and hh == 1) else nc.vector
                        eng.tensor_tensor(
                            out=yl[:, :, hh * PSC:(hh + 1) * PSC],
                            in0=src_ch[:, :, hh * PSC:(hh + 1) * PSC],
                            in1=yq[:, :, hh * PSC:(hh + 1) * PSC],
                            op=OP.subtract)
                else:
                    for hh in range(2):
                        nc.vector._custom_dve(
                            AFFINE_THEN_ADD,
                            out=yl[:, :, hh * PSC:(hh + 1) * PSC],
                            in0=yq[:, :, hh * PSC:(hh + 1) * PSC],
                            in1=src_ch[:, :, hh * PSC:(hh + 1) * PSC],
                            s0=-1.0, s1=0.0)
            return yq, yl

        # ================= wavefront-pipelined phases =================
        # Masters live as per-pair chunk tiles [128, KCH, PAIR] in an
        # 8-deep ring, so phase P+1's writes only WAR-depend on the same
        # chunk's readers (not the whole previous master).  Phases are
        # emitted in an interleaved wavefront so conv1's PE-heavy
        # iterations overlap the ACT/DVE-bound highway iterations.
        cvt = {}
        mch = {}

        def new_mchunk(name):
            return mst.tile([128, KCH, PAIR], BF16, tag="mst", name=name)

        def conv0_pair(pr):
            ch = new_mchunk(f"m0_{pr}")
            mch[(0, pr)] = ch
            base2 = pr * PAIR
            for m in range(MCH):
                mc = slice(m * 128, (m + 1) * 128)
                pc = ps.tile([128, PAIR], F32, tag="ps", name="pc")
                for half in range(2):
                    base = base2 + half * PSC
                    hs = half * PSC
                    if STACK0:
                        # taps 1+2 stacked over 128 partitions, then tap 0
                        nc.tensor.matmul(
                            out=pc[:, hs:hs + PSC],
                            lhsT=w0s_sb[:, mc],
                            rhs=xg[:, base:base + PSC],
                            start=True, stop=False)
                        lo, ln, o0 = base - 1, PSC, 0
                        if lo < 0:
                            lo, ln, o0 = 0, PSC - 1, 1
                        nc.tensor.matmul(
                            out=pc[:, hs + o0:hs + PSC],
                            lhsT=w0_sb[:, 0, mc],
                            rhs=xg[0:DB, lo:lo + ln],
                            start=False, stop=True)
                    else:
                        for j, k in enumerate((1, 0, 2)):
                            lo, ln = base + (k - 1), PSC
                            o0, o1 = 0, PSC
                            if lo < 0:
                                lo, ln, o0 = 0, PSC - 1, 1
                            elif lo + ln > T:
                                ln, o1 = T - lo, PSC - 1
                            nc.tensor.matmul(
                                out=pc[:, hs + o0:hs + o1],
                                lhsT=w0_sb[:, k, mc],
                                rhs=xg[0:DB, lo:lo + ln],
                                start=(j == 0), stop=(j == 2))
                evict_relu(ch[:, m, :], pc[:], m, "conv0", "conv0")
            cvt[(0, pr)] = emit_cvt(ch, 0 not in skip_lo, f"h0_{pr}")

        def hw_pair(L, pr, pin, pout, after=None):
            phase = "hw0" if L < 2 else ("hwy" if L == 2 else "hw1")
            use_lo = L not in skip_lo
            adn, addd = route[phase][2], route[phase][3]
            src = mch[(pin, pr)]
            dst = new_mchunk(f"mL{L}_{pr}")
            mch[(pout, pr)] = dst
            yq, yl = cvt[(L, pr)]
            for m in range(MCH):
                mc = slice(m * 128, (m + 1) * 128)
                pg = ps.tile([128, PAIR], F32, tag="ps", name="pg")
                for half in range(2):
                    hs = half * PSC
                    for k in range(2):
                        nc.tensor.matmul(
                            out=pg[:, hs:hs + PSC],
                            lhsT=wg_sb[:, L, 2 * k:2 * k + 2, mc],
                            rhs=yq[:, 2 * k:2 * k + 2, hs:hs + PSC],
                            start=(k == 0), stop=(k == 1),
                            perf_mode=PM.DoubleRow)
                phh = ps.tile([128, PAIR], F32, tag="ps", name="ph")
                srcs = [(whh_sb, yq), (whl_sb, yq)]
                if use_lo:
                    srcs.append((whh_sb, yl))
                nmm_h = 2 * len(srcs)
                for half in range(2):
                    hs = half * PSC
                    i = 0
                    for w_sb, rtns in srcs:
                        for k in range(2):
                            nc.tensor.matmul(
                                out=phh[:, hs:hs + PSC],
                                lhsT=w_sb[:, L, 2 * k:2 * k + 2, mc],
                                rhs=rtns[:, 2 * k:2 * k + 2, hs:hs + PSC],
                                start=(i == 0), stop=(i == nmm_h - 1),
                                perf_mode=PM.DoubleRow)
                            i += 1
                g = gp.tile([128, PAIR], BF16, tag="g", name="g")
                nc.scalar.activation(
                    out=g[:], in_=pg[:], func=AF.Sigmoid, scale=inv_ak,
                    bias=bias_sb[:, 4 + L * 4 + m:5 + L * 4 + m])
                dslc = dst[:, m, :]
                yslc = src[:, m, :]
                evict_relu(dslc, phh[:], m, "hw", phase)
                d = wkp.tile([128, PAIR], BF16, tag="d", name="d")
                nc.vector.tensor_tensor(
                    out=d[:], in0=dslc, in1=yslc, op=OP.subtract)
                last_pair = LAST_DVE and L == 3 and pr == NPR - 1
                if (m % addd) < adn and not last_pair:
                    nc.vector.tensor_tensor(
                        out=dslc, in0=g[:], in1=d[:], op=OP.mult)
                    nc.gpsimd.dma_start(out=dslc, in_=yslc,
                                        accum_op=OP.add)
                else:
                    nc.vector.tensor_tensor(
                        out=d[:], in0=g[:], in1=d[:], op=OP.mult)
                    nc.vector.tensor_tensor(
                        out=dslc, in0=yslc, in1=d[:], op=OP.add)
            if after is not None:
                after(dst, pr)

        def conv1_pair(pr):
            src = mch[(2, pr)]
            dst = new_mchunk(f"m1_{pr}")
            mch[(3, pr)] = dst
            base2 = pr * PAIR
            for m in range(MCH):
                mc = slice(m * 128, (m + 1) * 128)
                pc = ps.tile([128, PAIR], F32, tag="ps", name="pc1")
                for half in range(2):
                    base = base2 + half * PSC
                    hs = half * PSC
                    first = True
                    for k in (1, 0, 2):
                        lo, ln = base + (k - 1), PSC
                        o0 = 0
                        if lo < 0:
                            lo, ln, o0 = 0, PSC - 1, 1
                        elif lo + ln > T:
                            ln = T - lo
                        # split the source range across yq chunk tiles
                        segs = []
                        s = lo
                        while s < lo + ln:
                            chk = s // YQS
                            e = min(lo + ln, (chk + 1) * YQS)
                            segs.append((chk, s, e))
                            s = e
                        for w_sb, nkc in ((w1h_sb, 2),
                                          (w1l_sb, 1 if W1_HALF else 2)):
                            for kc in range(nkc):
                                for chk, s, e in segs:
                                    oo = hs + o0 + (s - lo)
                                    yt = cvt[("c1", chk)][0]
                                    nc.tensor.matmul(
                                        out=pc[:, oo:oo + (e - s)],
                                        lhsT=w_sb[:, k,
                                                  2 * kc:2 * kc + 2, mc],
                                        rhs=yt[:, 2 * kc:2 * kc + 2,
                                               s - chk * YQS:e - chk * YQS],
                                        start=first, stop=False,
                                        perf_mode=PM.DoubleRow)
                                    first = False
                    nc.tensor.matmul(
                        out=pc[:, hs:hs + PSC],
                        lhsT=id16_sb[:],
                        rhs=src[:, m, hs:hs + PSC],
                        start=False, stop=True)
                evict_relu(dst[:, m, :], pc[:], m, "hw", "conv1")
            cvt[(2, pr)] = emit_cvt(dst, 2 not in skip_lo, f"h2_{pr}")

        # ---- transposes / segmax / proj emitters ----
        def emit_transpose2(src_ch, pr, blk2):
            # two 128-token blocks per single-bank bf16 psum tile
            pt = ptp.tile([128, PAIR], BF16,
                          tag="pt" if ptp_bufs else "ps", name="pt")
            for c in range(2):
                lb = (blk2 * 2 + c) * 128 - pr * PAIR
                for m in range(MCH):
                    nc.tensor.transpose(
                        out=pt[:, c * PSC + m * 128:c * PSC + (m + 1) * 128],
                        in_=src_ch[:, m, lb:lb + 128],
                        identity=ident_sb[:])
            st = tp.tile([128, PAIR], BF16, tag="y1t", name="st")
            an, ad = route["tr"][0], route["tr"][1]
            if (blk2 % ad) < an:
                nc.scalar.activation(out=st[:], in_=pt[:], func=AF.Copy)
            else:
                nc.vector.tensor_copy(out=st[:], in_=pt[:])
            r0 = blk2 * 256
            if Y1T_SPLIT:
                for c in range(2):
                    nc.sync.dma_start(
                        out=y1t_d[r0 + c * 128:r0 + (c + 1) * 128, :],
                        in_=st[:, c * PSC:(c + 1) * PSC])
            else:
                nc.sync.dma_start(
                    out=y1t_d[r0:r0 + 256, :].rearrange(
                        "(c r) f -> r c f", c=2),
                    in_=st[:].rearrange("p (c f) -> p c f", c=2))

        a2_all = xap.tile([128, KCH, W], BF16, tag="xa", name="a2_all")

        taps = {}

        def emit_gather(wc):
            tap = gat.tile([128, KCH, ntaps * 128], BF16, tag="tap",
                           name="tap")
            taps[wc] = tap
            nc.gpsimd.dma_gather(
                out_ap=tap[:],
                in_ap=y1t_d[0:rmaxs[wc], :],
                idxs_ap=gidx_sb[:, wc * ntaps * 8:(wc + 1) * ntaps * 8],
                num_idxs=ntaps * 128,
                num_idxs_reg=ntaps * 128,
                elem_size=DW,
                transpose=True,
                single_packet=False,
            )

        def emit_tree(wc):
            tap = taps[wc]
            a2s = a2_all[:, :, wc * 128:(wc + 1) * 128]
            eng = nc.gpsimd if TREE_POOL else nc.vector
            # balanced max tree (depth 3 for ntaps=5)
            t1 = wkp.tile([128, KCH, 128], BF16, tag="t1", name="t1")
            eng.tensor_tensor(
                out=t1[:], in0=tap[:, :, 0:128], in1=tap[:, :, 128:256],
                op=OP.max)
            if ntaps >= 4:
                eng.tensor_tensor(
                    out=a2s, in0=tap[:, :, 256:384], in1=tap[:, :, 384:512],
                    op=OP.max)
                eng.tensor_tensor(out=a2s, in0=a2s, in1=t1[:], op=OP.max)
                for j in range(4, ntaps):
                    eng.tensor_tensor(
                        out=a2s, in0=a2s,
                        in1=tap[:, :, j * 128:(j + 1) * 128], op=OP.max)
            else:
                eng.tensor_tensor(
                    out=a2s, in0=t1[:], in1=tap[:, :, 256:384], op=OP.max)

        def emit_proj_wc(wc):
            if PROJ_ALT and wc % 2 == 1:
                po = ptp.tile([128, DW], F32, tag="pt", name="po")
            else:
                po = ppp.tile([128, DW], F32, tag="po", name="po")
            for k in range(KCH):
                nc.tensor.matmul(
                    out=po[:],
                    lhsT=a2_all[:, k, wc * 128:(wc + 1) * 128],
                    rhs=projw_sb[:, k, :],
                    start=(k == 0), stop=False)
            nc.tensor.matmul(
                out=po[:], lhsT=ones_sb[:, 0:128], rhs=projb_sb[:],
                start=False, stop=True)
            ob = obp.tile([128, DW], F32, tag="ob", name="ob")
            an, ad = route["proj"][0], route["proj"][1]
            if (wc % ad) < an:
                nc.scalar.activation(out=ob[:], in_=po[:], func=AF.Copy)
            else:
                nc.vector.tensor_copy(out=ob[:], in_=po[:])
            nc.sync.dma_start(
                out=out_d[wc * 128:(wc + 1) * 128, :], in_=ob[:])

        def emit_proj(wc2):
            po = ps.tile([128, PAIR], F32, tag="ps", name="po")
            for c in range(2):
                wc = wc2 * 2 + c
                for k in range(KCH):
                    nc.tensor.matmul(
                        out=po[:, c * PSC:c * PSC + DW],
                        lhsT=a2_all[:, k, wc * 128:(wc + 1) * 128],
                        rhs=projw_sb[:, k, :],
                        start=(k == 0), stop=False)
                nc.tensor.matmul(
                    out=po[:, c * PSC:c * PSC + DW],
                    lhsT=ones_sb[:, 0:128], rhs=projb_sb[:],
                    start=False, stop=True)
            ob = obp.tile([128, PAIR], F32, tag="ob", name="ob")
            an, ad = route["proj"][0], route["proj"][1]
            if (wc2 % ad) < an:
                nc.scalar.activation(out=ob[:], in_=po[:], func=AF.Copy)
            else:
                nc.vector.tensor_copy(out=ob[:], in_=po[:])
            r0 = wc2 * 256
            nc.sync.dma_start(
                out=out_d[r0:r0 + 256, :].rearrange(
                    "(c r) f -> r c f", c=2),
                in_=ob[:].rearrange("p (c f) -> p c f", c=2))

        # gather wc becomes legal after the L3 pair providing y1t row rmax
        pair_of_wc = [max(0, (rmaxs[wc] + PAIR - 1) // PAIR - 1)
                      for wc in range(8)]
        seg_done = [0]

        def ap0(dst, pr):
            cvt[(1, pr)] = emit_cvt(dst, 1 not in skip_lo, f"h1_{pr}")

        def ap1(dst, pr):
            cvt[("c1", pr)] = emit_cvt(dst, False, f"c1_{pr}", pool=c1p)

        def ap2(dst, pr):
            cvt[(3, pr)] = emit_cvt(dst, 3 not in skip_lo, f"h3_{pr}")

        def ap3(dst, pr):
            for blk2 in range(pr * 4, (pr + 1) * 4):
                emit_transpose2(dst, pr, blk2)
            if FOLD_SEG:
                while (seg_done[0] < min(8, FOLD_SEG)
                       and pair_of_wc[seg_done[0]] <= pr):
                    wc = seg_done[0]
                    emit_gather(wc)
                    seg_done[0] += 1

        # wavefront schedule: phase -> wave offset
        emitters = {
            0: conv0_pair,
            1: lambda pr: hw_pair(0, pr, 0, 1, after=ap0),
            2: lambda pr: hw_pair(1, pr, 1, 2, after=ap1),
            3: conv1_pair,
            4: lambda pr: hw_pair(2, pr, 3, 4, after=ap2),
            5: lambda pr: hw_pair(3, pr, 4, 5, after=ap3),
        }
        startw = dict(zip(range(6), phase_offsets))
        nwave = max(startw.values()) + NPR
        for wv in range(nwave):
            for P in range(6):
                pr = wv - startw[P]
                if 0 <= pr < NPR:
                    emitters[P](pr)

        if TAIL_WARM:
            for i in range(TAIL_WARM):
                wt = ptp.tile([128, PAIR], BF16, tag="pt", name="twarm")
                nc.tensor.transpose(out=wt[:, 0:128], in_=ident_sb[:],
                                    identity=ident_sb[:])
        for wc in range(8):
            if wc >= seg_done[0]:
                emit_gather(wc)
            emit_tree(wc)
            if FOLD_PROJ and wc % 2 == 1:
                emit_proj(wc // 2)
        if not FOLD_PROJ:
            if PROJ_PAIR:
                for wc2 in range(4):
                    emit_proj(wc2)
            else:
                for wc in range(8):
                    emit_proj_wc(wc)

    nc.compile()
    return nc


@functools.lru_cache(maxsize=4)
def _program(key) -> bass.Bass:
    ntaps, rmaxs = key
    return build_program(ntaps, rmaxs)


def _pack_idx(lin: np.ndarray) -> np.ndarray:
    n = len(lin)
    assert n % 16 == 0
    arr = np.asarray(lin, dtype=np.int16).reshape(n // 16, 16).T
    return np.tile(arr, (8, 1)).copy()


def prepare(inputs):
    f32 = np.float32
    bt = np.asarray(inputs["byte_tokens"]).astype(np.int64)
    bpe = np.asarray(inputs["bpe_mask"]).astype(np.int64)
    wrd = np.asarray(inputs["word_mask"]).astype(np.int64)
    seg = np.asarray(inputs["seg_ids"]).astype(np.int64)
    emb = np.asarray(inputs["tok_emb"], dtype=f32)
    conv0_w = np.asarray(inputs["conv0_w"], dtype=f32)
    conv0_b = np.asarray(inputs["conv0_b"], dtype=f32)
    conv1_w = np.asarray(inputs["conv1_w"], dtype=f32)
    conv1_b = np.asarray(inputs["conv1_b"], dtype=f32)
    hw_w = {(0, "g"): np.asarray(inputs["hw0_wg"], dtype=f32),
            (0, "h"): np.asarray(inputs["hw0_wh"], dtype=f32),
            (1, "g"): np.asarray(inputs["hw1_wg"], dtype=f32),
            (1, "h"): np.asarray(inputs["hw1_wh"], dtype=f32)}
    hw_b = {(0, "g"): np.asarray(inputs["hw0_bg"], dtype=f32),
            (0, "h"): np.asarray(inputs["hw0_bh"], dtype=f32),
            (1, "g"): np.asarray(inputs["hw1_bg"], dtype=f32),
            (1, "h"): np.asarray(inputs["hw1_bh"], dtype=f32)}
    proj_w = np.asarray(inputs["proj_w"], dtype=f32)
    proj_b = np.asarray(inputs["proj_b"], dtype=f32)

    assert np.all(conv1_b == 0) and all(np.all(hw_b[k] == 0)
                                        for k in ((0, "h"), (1, "h"))), \
        "nonzero h/conv1 biases unsupported by fast eviction path"

    def as_bf16(x):
        return np.ascontiguousarray(x.astype(bf16_np))

    def as_fp8(x):
        return np.ascontiguousarray(x.astype(fp8_np))

    embc = np.zeros((CVOCAB, 128), f32)
    for bm in range(2):
        for wm in range(2):
            r0 = VOCAB * (bm + 2 * wm)
            embc[r0:r0 + VOCAB, :DB] = (
                emb + bm * emb[BPE_MARK] + wm * emb[WORD_MARK])

    def chunk_kl(wm):
        L = wm.shape[0]
        return np.transpose(wm.reshape(L, KCH, 128, DW), (2, 0, 1, 3))

    wg_all = np.concatenate([hw_w[(0, "g")], hw_w[(1, "g")]], axis=0)
    wh_all = np.concatenate([hw_w[(0, "h")], hw_w[(1, "h")]], axis=0)
    wg_k = chunk_kl(K_SCALE * wg_all)
    wh_k = chunk_kl(K_SCALE * wh_all)
    wh_hi = wh_k.astype(fp8_np).astype(f32)
    wh_lo = wh_k - wh_hi
    w1_k = np.transpose((K_SCALE * conv1_w).reshape(3, KCH, 128, DW),
                        (2, 0, 1, 3))
    w1_hi = w1_k.astype(fp8_np).astype(f32)
    w1_lo = w1_k - w1_hi

    shared = {
        "emb_comb": as_bf16(embc),
        "w0": as_bf16(A_SCALE * np.transpose(conv0_w, (1, 0, 2))),
        "w0s": as_bf16(np.concatenate(
            [A_SCALE * conv0_w[1], A_SCALE * conv0_w[2]], axis=0)),
        "wg8": as_fp8(wg_k),
        "wh8": as_fp8(wh_hi),
        "wl8": as_fp8(wh_lo),
        "w1h8": as_fp8(w1_hi),
        "w1l8": as_fp8(w1_lo),
        "id16": np.ascontiguousarray(
            (K_SCALE * np.eye(128, dtype=f32)).astype(bf16_np)),
        "ident": np.eye(128, dtype=bf16_np),
        "projw": as_bf16(np.transpose((proj_w / A_SCALE).reshape(
            KCH, 128, DW), (1, 0, 2))),
        "projb": as_bf16(proj_b.reshape(1, DW)),
    }

    bias_h = np.zeros((128, 24), f32)
    bias_h[:, 0:4] = (A_SCALE * conv0_b).reshape(KCH, 128).T
    bg_all = np.concatenate([hw_b[(0, "g")], hw_b[(1, "g")]], axis=0)
    for L in range(4):
        bias_h[:, 4 + L * 4:8 + L * 4] = bg_all[L].reshape(KCH, 128).T
    shared["biases"] = np.ascontiguousarray(bias_h)

    counts = np.zeros((B, W), np.int64)
    for b in range(B):
        counts[b] = np.bincount(seg[b], minlength=W)[:W]
    assert (counts >= 1).all(), "empty segments unsupported"
    ntaps = max(int(counts.max()), 2)
    starts = np.zeros((B, W), np.int64)
    starts[:, 1:] = np.cumsum(counts, axis=1)[:, :-1]
    ends = starts + counts - 1

    in_maps = []
    gmax = np.zeros(8, np.int64)
    for b in range(B):
        cidx = bt[b] + VOCAB * (bpe[b] + 2 * wrd[b])
        gl = np.empty(8 * ntaps * 128, np.int64)
        for wc in range(8):
            nvec = np.arange(ntaps * 128)
            wv = wc * 128 + (nvec % 128)
            jv = nvec // 128
            sl = slice(wc * ntaps * 128, (wc + 1) * ntaps * 128)
            gl[sl] = np.minimum(starts[b, wv] + jv, ends[b, wv])
            gmax[wc] = max(gmax[wc], int(gl[sl].max()))
        m = dict(shared)
        m["tok_idx"] = _pack_idx(cidx)
        m["gidx"] = np.concatenate(
            [_pack_idx(gl[wc * ntaps * 128:(wc + 1) * ntaps * 128])
             for wc in range(8)], axis=1).copy()
        in_maps.append(m)
    rmaxs = tuple(int(min((g + 128) // 128 * 128, T)) for g in gmax)
    return (ntaps, rmaxs), in_maps


def _run(inputs, trace=False, **kwargs):
    key, in_maps = prepare(inputs)
    nc = _program(key)
    res = run_bass_kernel_spmd(
        nc, in_maps, core_ids=list(range(NCORES)), trace=trace, **kwargs)
    out = np.stack([res.results[b]["out"] for b in range(B)], axis=0)
    return out.astype(np.float32), res


def kernel(**inputs) -> np.ndarray:
    out, _ = _run(inputs, trace=False)
    return out


def run_traced(inputs, **kwargs):
    return _run(inputs, trace=True, **kwargs)


# revision 51
# speedup vs baseline: 1.0060x; 1.0060x over previous
"""Trainium2 Bass kernel for nn_ByteSequenceEmbedder — fp8 DoubleRow version.

Per core (1 sequence, 8 sequences data-parallel over 8 cores):
  x  = tok_emb[tokens] + bpe*E[4] + word*E[3]                 [T=4096, 64]
  x  = relu(conv3(x, W0) + b0); 2x highway(512)               [T, 512]
  x  = relu(conv3(x, W1) + b1 + x); 2x highway(512)           [T, 512]
  x  = per-word segment max (ragged, sorted seg_ids, W=1024)  [W, 512]
  out= x @ Pw + Pb                                            [W, 512]

Numerics (validated in numsim.py, rel err ~1.69e-2):
 - masters bf16, scaled by A=64, nonnegative
 - hw g-matmuls: plain fp8e4 DoubleRow (K=16-scaled weights)
 - hw h-matmuls: full hi/lo fp8 split (yq@Wh + yl@Wh + yq@Wl)
 - conv1: weight-split + bf16 identity tap (K*I) for the residual
 - conv0 / proj / transpose / segmax: bf16

Scheduling: evictions/combines at 1024-token pair granularity (2-bank
PSUM tiles); the embedding gathers go ahead of the (chunked) weight
DMAs; PE p-state is pre-warmed with dummy transposes and the act table
preloaded with a dummy sigmoid; each phase emits the next phase's
fp8 converts per-chunk so they pipeline instead of serializing at
phase boundaries; segmax gathers use tight per-chunk row bounds.
"""

import functools
import os
import sys

import numpy as np

for _p in ("/opt/trn_rl_repo", "/root/.axon_site/_ro/trn_rl_repo"):
    if os.path.isdir(_p) and _p not in sys.path:
        sys.path.append(_p)

import ml_dtypes  # noqa: E402

from contextlib import ExitStack  # noqa: E402

from concourse import bacc, bass, mybir, tile  # noqa: E402
from concourse import library_config  # noqa: E402
from concourse.bass_utils import run_bass_kernel_spmd  # noqa: E402
from concourse.dve_ops import AFFINE_THEN_ADD  # noqa: E402

B, T, W = 8, 4096, 1024
DB, DW = 64, 512
NH = 2
VOCAB = 264
BPE_MARK, WORD_MARK = 4, 3
PSC = 512          # tokens per psum bank
PAIR = 1024        # tokens per paired psum tile (2 banks)
NPR = T // PAIR    # 4 pairs
YQS = 1024         # tokens per yq/yl chunk == PAIR
MCH = DW // 128
KCH = DW // 128
NCORES = 8
CVOCAB = 4 * VOCAB
A_SCALE = 64.0
K_SCALE = 16.0
GCH = (1024, 2048, 3072, 4096)   # gather chunk boundaries

BF16 = mybir.dt.bfloat16
F32 = mybir.dt.float32
FP8 = mybir.dt.float8e4
I16 = mybir.dt.int16
AF = mybir.ActivationFunctionType
OP = mybir.AluOpType
PM = mybir.MatmulPerfMode

bf16_np = ml_dtypes.bfloat16
fp8_np = ml_dtypes.float8_e4m3


def build_program(ntaps: int, rmaxs: tuple, stage: int = 9,
                  skip_lo: tuple = (0, 1), warm: int = 40,
                  ps_bufs: int = 3, ptp_bufs: int = 1,
                  fold_seg: int = 0, fold_proj: bool = False,
                  tail_warm: int = 0, y1t_split: bool = False,
                  proj_alt: bool = False, cvt_sync: bool = False,
                  stack0: bool = True, last_dve: bool = False,
                  unstack0: int = 1,
                  gp_bufs: int = 3, wkp_bufs: int = 3,
                  tp_bufs: int = 3, c1p_bufs: int = 4, ylp_bufs: int = 4,
                  phase_offsets: tuple = (0, 2, 4, 7, 9, 11),
                  tree_pool: bool = False, gat_bufs: int = 3,
                  cvt_halves: bool = False, yl_tt: bool = True,
                  yl_pool: bool = True, w1_half: bool = True,
                  proj_pair: bool = False,
                  yqp_bufs: int = 4, mst_bufs: int = 8,
                  routing: dict | None = None) -> bass.Bass:
    FOLD_SEG, FOLD_PROJ = int(fold_seg), fold_proj
    TAIL_WARM = tail_warm
    Y1T_SPLIT = y1t_split
    PROJ_ALT = proj_alt
    CVT_SYNC = cvt_sync
    STACK0 = stack0
    LAST_DVE = last_dve
    UNSTACK = unstack0
    TREE_POOL = tree_pool
    CVT_HALVES = cvt_halves
    YL_TT = yl_tt
    YL_POOL = yl_pool
    W1_HALF = w1_half
    PROJ_PAIR = proj_pair
    nc = bacc.Bacc("TRN2", target_bir_lowering=False, debug=False)

    def din(name, shape, dtype):
        return nc.dram_tensor(name, list(shape), dtype, kind="ExternalInput")

    emb_d = din("emb_comb", (CVOCAB, 128), BF16)
    tokidx_d = din("tok_idx", (128, T // 16), I16)
    w0_d = din("w0", (DB, 3, DW), BF16)
    w0s_d = din("w0s", (128, DW), BF16)
    wg_d = din("wg8", (128, 4, KCH, DW), FP8)
    whh_d = din("wh8", (128, 4, KCH, DW), FP8)
    whl_d = din("wl8", (128, 4, KCH, DW), FP8)
    w1h_d = din("w1h8", (128, 3, KCH, DW), FP8)
    w1l_d = din("w1l8", (128, 3, KCH, DW), FP8)
    id16_d = din("id16", (128, 128), BF16)
    ident_d = din("ident", (128, 128), BF16)
    projw_d = din("projw", (128, KCH, DW), BF16)
    projb_d = din("projb", (1, DW), BF16)
    bias_d = din("biases", (128, 24), F32)
    gidx_d = din("gidx", (128, 8 * ntaps * 8), I16)
    out_d = nc.dram_tensor("out", [W, DW], F32, kind="ExternalOutput")
    y1t_d = nc.dram_tensor("y1t", [T, DW], BF16, kind="Internal")

    inv_ak = 1.0 / (A_SCALE * K_SCALE)
    inv_k = 1.0 / K_SCALE

    with tile.TileContext(nc) as tc, ExitStack() as ctx:
        const = ctx.enter_context(tc.tile_pool(name="const", bufs=1))
        xap = ctx.enter_context(tc.tile_pool(name="xap", bufs=1))
        ps = ctx.enter_context(tc.tile_pool(name="psp", bufs=ps_bufs,
                                            space="PSUM"))
        ptp = (ctx.enter_context(tc.tile_pool(name="ptp", bufs=ptp_bufs,
                                              space="PSUM"))
               if ptp_bufs else ps)
        ppp = ctx.enter_context(tc.tile_pool(name="ppp", bufs=1,
                                             space="PSUM"))

        mst = ctx.enter_context(tc.tile_pool(name="mst", bufs=mst_bufs))
        yqp = ctx.enter_context(tc.tile_pool(name="yqp", bufs=yqp_bufs))
        c1p = ctx.enter_context(tc.tile_pool(name="c1p", bufs=c1p_bufs))
        ylp = ctx.enter_context(tc.tile_pool(name="ylp", bufs=ylp_bufs))
        gp = ctx.enter_context(tc.tile_pool(name="gpool", bufs=gp_bufs))
        wkp = ctx.enter_context(tc.tile_pool(name="wkp", bufs=wkp_bufs))
        tp = ctx.enter_context(tc.tile_pool(name="tpool", bufs=tp_bufs))
        gat = ctx.enter_context(tc.tile_pool(name="gat", bufs=gat_bufs))
        obp = ctx.enter_context(tc.tile_pool(name="obp", bufs=2))

        nc.gpsimd.load_library(library_config.mlp)

        def load(dram_t, shape, dtype, name, src=None):
            t = const.tile(shape, dtype, name=name)
            nc.sync.dma_start(out=t[:], in_=src if src is not None
                              else dram_t[:])
            return t

        # small consts on the SP queue, in consumption order
        tokidx_sb = load(tokidx_d, [128, T // 16], I16, "tokidx_sb")
        ident_sb = load(ident_d, [128, 128], BF16, "ident_sb")
        w0_sb = load(w0_d, [DB, 3, DW], BF16, "w0_sb")
        w0s_sb = load(w0s_d, [128, DW], BF16, "w0s_sb")
        bias_sb = load(bias_d, [128, 24], F32, "bias_sb")

        # preload the activation table set that serves sigmoid+relu+copy
        sigscr = const.tile([1, 8], F32, name="sigscr")
        nc.scalar.activation(out=sigscr[:], in_=bias_sb[0:1, 0:8],
                             func=AF.Sigmoid)

        # PE warmup: dummy transposes to ramp the p-state before conv0
        if warm:
            wps = ptp.tile([128, PAIR], BF16,
                           tag="pt" if ptp_bufs else "ps", name="warm")
            for i in range(warm):
                nc.tensor.transpose(
                    out=wps[:, 0:128],
                    in_=ident_sb[:], identity=ident_sb[:])

        # ---- embedding gather: ahead of the big weight DMAs ----
        xg = xap.tile([128, T], BF16, tag="xa", name="xg")
        if stage >= 1:
            prev = 0
            for r, hi in enumerate(GCH):
                ec = hi - prev
                nc.gpsimd.dma_gather(
                    out_ap=xg[:, prev:hi].rearrange(
                        "p (c n) -> p c n", c=1),
                    in_ap=emb_d[:],
                    idxs_ap=tokidx_sb[:, prev // 16:hi // 16],
                    num_idxs=ec,
                    num_idxs_reg=ec,
                    elem_size=128,
                    transpose=True,
                    single_packet=False,
                )
                if STACK0:
                    # xg[64:128, t] = xg[0:64, t+1]; col T-1 keeps zeros
                    lo = max(prev - 1, 0)
                    nc.sync.dma_start(
                        out=xg[64:128, lo:hi - 1],
                        in_=xg[0:DB, lo + 1:hi])
                prev = hi
        else:
            nc.vector.memset(xg[:], 0.0)

        # big weights: chunked DMAs on the SP queue so individual
        # transfers interleave with (rather than blocking) the gathers
        def loadc(dram_t, shape, dtype, name, nch):
            t = const.tile(shape, dtype, name=name)
            for c in range(nch):
                sl = slice(c * shape[1] // nch, (c + 1) * shape[1] // nch)
                nc.sync.dma_start(out=t[:, sl], in_=dram_t[:, sl])
            return t

        wg_sb = loadc(wg_d, [128, 4, KCH, DW], FP8, "wg_sb", 4)
        whh_sb = loadc(whh_d, [128, 4, KCH, DW], FP8, "whh_sb", 4)
        whl_sb = loadc(whl_d, [128, 4, KCH, DW], FP8, "whl_sb", 4)
        w1h_sb = loadc(w1h_d, [128, 3, KCH, DW], FP8, "w1h_sb", 3)
        w1l_sb = loadc(w1l_d, [128, 3, KCH, DW], FP8, "w1l_sb", 3)
        id16_sb = load(id16_d, [128, 128], BF16, "id16_sb")
        projw_sb = loadc(projw_d, [128, KCH, DW], BF16, "projw_sb", 2)
        projb_sb = load(projb_d, [1, DW], BF16, "projb_sb")
        gidx_sb = load(gidx_d, [128, 8 * ntaps * 8], I16, "gidx_sb")
        ones_sb = const.tile([1, 128], BF16, name="ones_sb")
        nc.vector.memset(ones_sb[:], 1.0)

        # routing[phase] = (act_num, act_den, add_swdge_num, add_den):
        # fraction act_num/act_den of relu evictions go to ACT (rest DVE);
        # fraction add_swdge_num/add_den of combine adds go to SWDGE.
        route = {"conv0": (2, 4, 0, 1), "hw0": (2, 3, 4, 4),
              "hw1": (3, 4, 3, 4), "conv1": (3, 4, 0, 1),
              "hwy": (3, 4, 3, 4), "tr": (1, 4, 0, 1), "proj": (4, 4, 0, 1)}
        if routing:
            route.update(routing)
        ev_ct = {}

        def evict_relu(dst, pc, m, scale_mode, phase):
            """dst = relu(pc*scale + bias) routed ACT/DVE per phase."""
            n = ev_ct.get(phase, 0)
            ev_ct[phase] = n + 1
            an, ad = route[phase][0], route[phase][1]
            use_act = (n % ad) < an
            if scale_mode == "conv0":
                if use_act:
                    nc.scalar.activation(
                        out=dst, in_=pc, func=AF.Relu,
                        bias=bias_sb[:, m:m + 1])
                else:
                    nc.vector.tensor_scalar(
                        out=dst, in0=pc,
                        scalar1=bias_sb[:, m:m + 1], scalar2=0.0,
                        op0=OP.add, op1=OP.max)
            else:  # inv_k scaled relu, zero bias
                if use_act:
                    nc.scalar.activation(
                        out=dst, in_=pc, func=AF.Relu, scale=inv_k,
                        bias=bias_sb[:, 20 + m:21 + m])
                else:
                    nc.vector.tensor_scalar(
                        out=dst, in0=pc, scalar1=0.0, scalar2=inv_k,
                        op0=OP.max, op1=OP.mult)

        def emit_cvt(src_ch, want_lo, lname, pool=None):
            yq = (pool or yqp).tile([128, KCH, YQS], FP8, tag="yq",
                                    name=f"yq{lname}")
            cq = nc.sync if CVT_SYNC else nc.gpsimd
            if CVT_HALVES:
                for hh in range(2):
                    cq.dma_start(
                        out=yq[:, :, hh * PSC:(hh + 1) * PSC],
                        in_=src_ch[:, :, hh * PSC:(hh + 1) * PSC])
            else:
                cq.dma_start(out=yq[:], in_=src_ch[:, :, :])
            yl = None
            if want_lo:
                yl = ylp.tile([128, KCH, YQS], FP8, tag="yl",
                              name=f"yl{lname}")
                if YL_TT:
                    for hh in range(2):
                        eng = nc.gpsimd if (YL_POOL and hh == 1) else nc.vector
                        eng.tensor_tensor(
                            out=yl[:, :, hh * PSC:(hh + 1) * PSC],
                            in0=src_ch[:, :, hh * PSC:(hh + 1) * PSC],
                            in1=yq[:, :, hh * PSC:(hh + 1) * PSC],
                            op=OP.subtract)
                else:
                    for hh in range(2):
                        nc.vector._custom_dve(
                            AFFINE_THEN_ADD,
                            out=yl[:, :, hh * PSC:(hh + 1) * PSC],
                            in0=yq[:, :, hh * PSC:(hh + 1) * PSC],
                            in1=src_ch[:, :, hh * PSC:(hh + 1) * PSC],
                            s0=-1.0, s1=0.0)
            return yq, yl

        # ================= wavefront-pipelined phases =================
        # Masters live as per-pair chunk tiles [128, KCH, PAIR] in an
        # 8-deep ring, so phase P+1's writes only WAR-depend on the same
        # chunk's readers (not the whole previous master).  Phases are
        # emitted in an interleaved wavefront so conv1's PE-heavy
        # iterations overlap the ACT/DVE-bound highway iterations.
        cvt = {}
        mch = {}

        def new_mchunk(name):
            return mst.tile([128, KCH, PAIR], BF16, tag="mst", name=name)

        def conv0_pair(pr):
            ch = new_mchunk(f"m0_{pr}")
            mch[(0, pr)] = ch
            base2 = pr * PAIR
            for m in range(MCH):
                mc = slice(m * 128, (m + 1) * 128)
                pc = ps.tile([128, PAIR], F32, tag="ps", name="pc")
                for half in range(2):
                    base = base2 + half * PSC
                    hs = half * PSC
                    if STACK0 and pr >= UNSTACK:
                        # taps 1+2 stacked over 128 partitions, then tap 0
                        nc.tensor.matmul(
                            out=pc[:, hs:hs + PSC],
                            lhsT=w0s_sb[:, mc],
                            rhs=xg[:, base:base + PSC],
                            start=True, stop=False)
                        lo, ln, o0 = base - 1, PSC, 0
                        if lo < 0:
                            lo, ln, o0 = 0, PSC - 1, 1
                        nc.tensor.matmul(
                            out=pc[:, hs + o0:hs + PSC],
                            lhsT=w0_sb[:, 0, mc],
                            rhs=xg[0:DB, lo:lo + ln],
                            start=False, stop=True)
                    else:
                        for j, k in enumerate((1, 0, 2)):
                            lo, ln = base + (k - 1), PSC
                            o0, o1 = 0, PSC
                            if lo < 0:
                                lo, ln, o0 = 0, PSC - 1, 1
                            elif lo + ln > T:
                                ln, o1 = T - lo, PSC - 1
                            nc.tensor.matmul(
                                out=pc[:, hs + o0:hs + o1],
                                lhsT=w0_sb[:, k, mc],
                                rhs=xg[0:DB, lo:lo + ln],
                                start=(j == 0), stop=(j == 2))
                evict_relu(ch[:, m, :], pc[:], m, "conv0", "conv0")
            cvt[(0, pr)] = emit_cvt(ch, 0 not in skip_lo, f"h0_{pr}")

        def hw_pair(L, pr, pin, pout, after=None):
            phase = "hw0" if L < 2 else ("hwy" if L == 2 else "hw1")
            use_lo = L not in skip_lo
            adn, addd = route[phase][2], route[phase][3]
            src = mch[(pin, pr)]
            dst = new_mchunk(f"mL{L}_{pr}")
            mch[(pout, pr)] = dst
            yq, yl = cvt[(L, pr)]
            for m in range(MCH):
                mc = slice(m * 128, (m + 1) * 128)
                pg = ps.tile([128, PAIR], F32, tag="ps", name="pg")
                for half in range(2):
                    hs = half * PSC
                    for k in range(2):
                        nc.tensor.matmul(
                            out=pg[:, hs:hs + PSC],
                            lhsT=wg_sb[:, L, 2 * k:2 * k + 2, mc],
                            rhs=yq[:, 2 * k:2 * k + 2, hs:hs + PSC],
                            start=(k == 0), stop=(k == 1),
                            perf_mode=PM.DoubleRow)
                phh = ps.tile([128, PAIR], F32, tag="ps", name="ph")
                srcs = [(whh_sb, yq), (whl_sb, yq)]
                if use_lo:
                    srcs.append((whh_sb, yl))
                nmm_h = 2 * len(srcs)
                for half in range(2):
                    hs = half * PSC
                    i = 0
                    for w_sb, rtns in srcs:
                        for k in range(2):
                            nc.tensor.matmul(
                                out=phh[:, hs:hs + PSC],
                                lhsT=w_sb[:, L, 2 * k:2 * k + 2, mc],
                                rhs=rtns[:, 2 * k:2 * k + 2, hs:hs + PSC],
                                start=(i == 0), stop=(i == nmm_h - 1),
                                perf_mode=PM.DoubleRow)
                            i += 1
                g = gp.tile([128, PAIR], BF16, tag="g", name="g")
                nc.scalar.activation(
                    out=g[:], in_=pg[:], func=AF.Sigmoid, scale=inv_ak,
                    bias=bias_sb[:, 4 + L * 4 + m:5 + L * 4 + m])
                dslc = dst[:, m, :]
                yslc = src[:, m, :]
                evict_relu(dslc, phh[:], m, "hw", phase)
                d = wkp.tile([128, PAIR], BF16, tag="d", name="d")
                nc.vector.tensor_tensor(
                    out=d[:], in0=dslc, in1=yslc, op=OP.subtract)
                last_pair = LAST_DVE and L == 3 and pr == NPR - 1
                if (m % addd) < adn and not last_pair:
                    nc.vector.tensor_tensor(
                        out=dslc, in0=g[:], in1=d[:], op=OP.mult)
                    nc.gpsimd.dma_start(out=dslc, in_=yslc,
                                        accum_op=OP.add)
                else:
                    nc.vector.tensor_tensor(
                        out=d[:], in0=g[:], in1=d[:], op=OP.mult)
                    nc.vector.tensor_tensor(
                        out=dslc, in0=yslc, in1=d[:], op=OP.add)
            if after is not None:
                after(dst, pr)

        def conv1_pair(pr):
            src = mch[(2, pr)]
            dst = new_mchunk(f"m1_{pr}")
            mch[(3, pr)] = dst
            base2 = pr * PAIR
            for m in range(MCH):
                mc = slice(m * 128, (m + 1) * 128)
                pc = ps.tile([128, PAIR], F32, tag="ps", name="pc1")
                for half in range(2):
                    base = base2 + half * PSC
                    hs = half * PSC
                    first = True
                    for k in (1, 0, 2):
                        lo, ln = base + (k - 1), PSC
                        o0 = 0
                        if lo < 0:
                            lo, ln, o0 = 0, PSC - 1, 1
                        elif lo + ln > T:
                            ln = T - lo
                        # split the source range across yq chunk tiles
                        segs = []
                        s = lo
                        while s < lo + ln:
                            chk = s // YQS
                            e = min(lo + ln, (chk + 1) * YQS)
                            segs.append((chk, s, e))
                            s = e
                        for w_sb, nkc in ((w1h_sb, 2),
                                          (w1l_sb, 1 if W1_HALF else 2)):
                            for kc in range(nkc):
                                for chk, s, e in segs:
                                    oo = hs + o0 + (s - lo)
                                    yt = cvt[("c1", chk)][0]
                                    nc.tensor.matmul(
                                        out=pc[:, oo:oo + (e - s)],
                                        lhsT=w_sb[:, k,
                                                  2 * kc:2 * kc + 2, mc],
                                        rhs=yt[:, 2 * kc:2 * kc + 2,
                                               s - chk * YQS:e - chk * YQS],
                                        start=first, stop=False,
                                        perf_mode=PM.DoubleRow)
                                    first = False
                    nc.tensor.matmul(
                        out=pc[:, hs:hs + PSC],
                        lhsT=id16_sb[:],
                        rhs=src[:, m, hs:hs + PSC],
                        start=False, stop=True)
                evict_relu(dst[:, m, :], pc[:], m, "hw", "conv1")
            cvt[(2, pr)] = emit_cvt(dst, 2 not in skip_lo, f"h2_{pr}")

        # ---- transposes / segmax / proj emitters ----
        def emit_transpose2(src_ch, pr, blk2):
            # two 128-token blocks per single-bank bf16 psum tile
            pt = ptp.tile([128, PAIR], BF16,
                          tag="pt" if ptp_bufs else "ps", name="pt")
            for c in range(2):
                lb = (blk2 * 2 + c) * 128 - pr * PAIR
                for m in range(MCH):
                    nc.tensor.transpose(
                        out=pt[:, c * PSC + m * 128:c * PSC + (m + 1) * 128],
                        in_=src_ch[:, m, lb:lb + 128],
                        identity=ident_sb[:])
            st = tp.tile([128, PAIR], BF16, tag="y1t", name="st")
            an, ad = route["tr"][0], route["tr"][1]
            if (blk2 % ad) < an:
                nc.scalar.activation(out=st[:], in_=pt[:], func=AF.Copy)
            else:
                nc.vector.tensor_copy(out=st[:], in_=pt[:])
            r0 = blk2 * 256
            if Y1T_SPLIT:
                for c in range(2):
                    nc.sync.dma_start(
                        out=y1t_d[r0 + c * 128:r0 + (c + 1) * 128, :],
                        in_=st[:, c * PSC:(c + 1) * PSC])
            else:
                nc.sync.dma_start(
                    out=y1t_d[r0:r0 + 256, :].rearrange(
                        "(c r) f -> r c f", c=2),
                    in_=st[:].rearrange("p (c f) -> p c f", c=2))

        a2_all = xap.tile([128, KCH, W], BF16, tag="xa", name="a2_all")

        taps = {}

        def emit_gather(wc):
            tap = gat.tile([128, KCH, ntaps * 128], BF16, tag="tap",
                           name="tap")
            taps[wc] = tap
            nc.gpsimd.dma_gather(
                out_ap=tap[:],
                in_ap=y1t_d[0:rmaxs[wc], :],
                idxs_ap=gidx_sb[:, wc * ntaps * 8:(wc + 1) * ntaps * 8],
                num_idxs=ntaps * 128,
                num_idxs_reg=ntaps * 128,
                elem_size=DW,
                transpose=True,
                single_packet=False,
            )

        def emit_tree(wc):
            tap = taps[wc]
            a2s = a2_all[:, :, wc * 128:(wc + 1) * 128]
            eng = nc.gpsimd if TREE_POOL else nc.vector
            # balanced max tree (depth 3 for ntaps=5)
            t1 = wkp.tile([128, KCH, 128], BF16, tag="t1", name="t1")
            eng.tensor_tensor(
                out=t1[:], in0=tap[:, :, 0:128], in1=tap[:, :, 128:256],
                op=OP.max)
            if ntaps >= 4:
                eng.tensor_tensor(
                    out=a2s, in0=tap[:, :, 256:384], in1=tap[:, :, 384:512],
                    op=OP.max)
                eng.tensor_tensor(out=a2s, in0=a2s, in1=t1[:], op=OP.max)
                for j in range(4, ntaps):
                    eng.tensor_tensor(
                        out=a2s, in0=a2s,
                        in1=tap[:, :, j * 128:(j + 1) * 128], op=OP.max)
            else:
                eng.tensor_tensor(
                    out=a2s, in0=t1[:], in1=tap[:, :, 256:384], op=OP.max)

        def emit_proj_wc(wc):
            if PROJ_ALT and wc % 2 == 1:
                po = ptp.tile([128, DW], F32, tag="pt", name="po")
            else:
                po = ppp.tile([128, DW], F32, tag="po", name="po")
            for k in range(KCH):
                nc.tensor.matmul(
                    out=po[:],
                    lhsT=a2_all[:, k, wc * 128:(wc + 1) * 128],
                    rhs=projw_sb[:, k, :],
                    start=(k == 0), stop=False)
            nc.tensor.matmul(
                out=po[:], lhsT=ones_sb[:, 0:128], rhs=projb_sb[:],
                start=False, stop=True)
            ob = obp.tile([128, DW], F32, tag="ob", name="ob")
            an, ad = route["proj"][0], route["proj"][1]
            if (wc % ad) < an:
                nc.scalar.activation(out=ob[:], in_=po[:], func=AF.Copy)
            else:
                nc.vector.tensor_copy(out=ob[:], in_=po[:])
            nc.sync.dma_start(
                out=out_d[wc * 128:(wc + 1) * 128, :], in_=ob[:])

        def emit_proj(wc2):
            po = ps.tile([128, PAIR], F32, tag="ps", name="po")
            for c in range(2):
                wc = wc2 * 2 + c
                for k in range(KCH):
                    nc.tensor.matmul(
                        out=po[:, c * PSC:c * PSC + DW],
                        lhsT=a2_all[:, k, wc * 128:(wc + 1) * 128],
                        rhs=projw_sb[:, k, :],
                        start=(k == 0), stop=False)
                nc.tensor.matmul(
                    out=po[:, c * PSC:c * PSC + DW],
                    lhsT=ones_sb[:, 0:128], rhs=projb_sb[:],
                    start=False, stop=True)
            ob = obp.tile([128, PAIR], F32, tag="ob", name="ob")
            an, ad = route["proj"][0], route["proj"][1]
            if (wc2 % ad) < an:
                nc.scalar.activation(out=ob[:], in_=po[:], func=AF.Copy)
            else:
                nc.vector.tensor_copy(out=ob[:], in_=po[:])
            r0 = wc2 * 256
            nc.sync.dma_start(
                out=out_d[r0:r0 + 256, :].rearrange(
                    "(c r) f -> r c f", c=2),
                in_=ob[:].rearrange("p (c f) -> p c f", c=2))

        # gather wc becomes legal after the L3 pair providing y1t row rmax
        pair_of_wc = [max(0, (rmaxs[wc] + PAIR - 1) // PAIR - 1)
                      for wc in range(8)]
        seg_done = [0]

        def ap0(dst, pr):
            cvt[(1, pr)] = emit_cvt(dst, 1 not in skip_lo, f"h1_{pr}")

        def ap1(dst, pr):
            cvt[("c1", pr)] = emit_cvt(dst, False, f"c1_{pr}", pool=c1p)

        def ap2(dst, pr):
            cvt[(3, pr)] = emit_cvt(dst, 3 not in skip_lo, f"h3_{pr}")

        def ap3(dst, pr):
            for blk2 in range(pr * 4, (pr + 1) * 4):
                emit_transpose2(dst, pr, blk2)
            if FOLD_SEG:
                while (seg_done[0] < min(8, FOLD_SEG)
                       and pair_of_wc[seg_done[0]] <= pr):
                    wc = seg_done[0]
                    emit_gather(wc)
                    seg_done[0] += 1

        # wavefront schedule: phase -> wave offset
        emitters = {
            0: conv0_pair,
            1: lambda pr: hw_pair(0, pr, 0, 1, after=ap0),
            2: lambda pr: hw_pair(1, pr, 1, 2, after=ap1),
            3: conv1_pair,
            4: lambda pr: hw_pair(2, pr, 3, 4, after=ap2),
            5: lambda pr: hw_pair(3, pr, 4, 5, after=ap3),
        }
        startw = dict(zip(range(6), phase_offsets))
        nwave = max(startw.values()) + NPR
        for wv in range(nwave):
            for P in range(6):
                pr = wv - startw[P]
                if 0 <= pr < NPR:
                    emitters[P](pr)

        if TAIL_WARM:
            for i in range(TAIL_WARM):
                wt = ptp.tile([128, PAIR], BF16, tag="pt", name="twarm")
                nc.tensor.transpose(out=wt[:, 0:128], in_=ident_sb[:],
                                    identity=ident_sb[:])
        for wc in range(8):
            if wc >= seg_done[0]:
                emit_gather(wc)
            emit_tree(wc)
            if FOLD_PROJ and wc % 2 == 1:
                emit_proj(wc // 2)
        if not FOLD_PROJ:
            if PROJ_PAIR:
                for wc2 in range(4):
                    emit_proj(wc2)
            else:
                for wc in range(8):
                    emit_proj_wc(wc)

    nc.compile()
    return nc


@functools.lru_cache(maxsize=4)
def _program(key) -> bass.Bass:
    ntaps, rmaxs = key
    return build_program(ntaps, rmaxs)


def _pack_idx(lin: np.ndarray) -> np.ndarray:
    n = len(lin)
    assert n % 16 == 0
    arr = np.asarray(lin, dtype=np.int16).reshape(n // 16, 16).T
    return np.tile(arr, (8, 1)).copy()


def prepare(inputs):
    f32 = np.float32
    bt = np.asarray(inputs["byte_tokens"]).astype(np.int64)
    bpe = np.asarray(inputs["bpe_mask"]).astype(np.int64)
    wrd = np.asarray(inputs["word_mask"]).astype(np.int64)
    seg = np.asarray(inputs["seg_ids"]).astype(np.int64)
    emb = np.asarray(inputs["tok_emb"], dtype=f32)
    conv0_w = np.asarray(inputs["conv0_w"], dtype=f32)
    conv0_b = np.asarray(inputs["conv0_b"], dtype=f32)
    conv1_w = np.asarray(inputs["conv1_w"], dtype=f32)
    conv1_b = np.asarray(inputs["conv1_b"], dtype=f32)
    hw_w = {(0, "g"): np.asarray(inputs["hw0_wg"], dtype=f32),
            (0, "h"): np.asarray(inputs["hw0_wh"], dtype=f32),
            (1, "g"): np.asarray(inputs["hw1_wg"], dtype=f32),
            (1, "h"): np.asarray(inputs["hw1_wh"], dtype=f32)}
    hw_b = {(0, "g"): np.asarray(inputs["hw0_bg"], dtype=f32),
            (0, "h"): np.asarray(inputs["hw0_bh"], dtype=f32),
            (1, "g"): np.asarray(inputs["hw1_bg"], dtype=f32),
            (1, "h"): np.asarray(inputs["hw1_bh"], dtype=f32)}
    proj_w = np.asarray(inputs["proj_w"], dtype=f32)
    proj_b = np.asarray(inputs["proj_b"], dtype=f32)

    assert np.all(conv1_b == 0) and all(np.all(hw_b[k] == 0)
                                        for k in ((0, "h"), (1, "h"))), \
        "nonzero h/conv1 biases unsupported by fast eviction path"

    def as_bf16(x):
        return np.ascontiguousarray(x.astype(bf16_np))

    def as_fp8(x):
        return np.ascontiguousarray(x.astype(fp8_np))

    embc = np.zeros((CVOCAB, 128), f32)
    for bm in range(2):
        for wm in range(2):
            r0 = VOCAB * (bm + 2 * wm)
            embc[r0:r0 + VOCAB, :DB] = (
                emb + bm * emb[BPE_MARK] + wm * emb[WORD_MARK])

    def chunk_kl(wm):
        L = wm.shape[0]
        return np.transpose(wm.reshape(L, KCH, 128, DW), (2, 0, 1, 3))

    wg_all = np.concatenate([hw_w[(0, "g")], hw_w[(1, "g")]], axis=0)
    wh_all = np.concatenate([hw_w[(0, "h")], hw_w[(1, "h")]], axis=0)
    wg_k = chunk_kl(K_SCALE * wg_all)
    wh_k = chunk_kl(K_SCALE * wh_all)
    wh_hi = wh_k.astype(fp8_np).astype(f32)
    wh_lo = wh_k - wh_hi
    w1_k = np.transpose((K_SCALE * conv1_w).reshape(3, KCH, 128, DW),
                        (2, 0, 1, 3))
    w1_hi = w1_k.astype(fp8_np).astype(f32)
    w1_lo = w1_k - w1_hi

    shared = {
        "emb_comb": as_bf16(embc),
        "w0": as_bf16(A_SCALE * np.transpose(conv0_w, (1, 0, 2))),
        "w0s": as_bf16(np.concatenate(
            [A_SCALE * conv0_w[1], A_SCALE * conv0_w[2]], axis=0)),
        "wg8": as_fp8(wg_k),
        "wh8": as_fp8(wh_hi),
        "wl8": as_fp8(wh_lo),
        "w1h8": as_fp8(w1_hi),
        "w1l8": as_fp8(w1_lo),
        "id16": np.ascontiguousarray(
            (K_SCALE * np.eye(128, dtype=f32)).astype(bf16_np)),
        "ident": np.eye(128, dtype=bf16_np),
        "projw": as_bf16(np.transpose((proj_w / A_SCALE).reshape(
            KCH, 128, DW), (1, 0, 2))),
        "projb": as_bf16(proj_b.reshape(1, DW)),
    }

    bias_h = np.zeros((128, 24), f32)
    bias_h[:, 0:4] = (A_SCALE * conv0_b).reshape(KCH, 128).T
    bg_all = np.concatenate([hw_b[(0, "g")], hw_b[(1, "g")]], axis=0)
    for L in range(4):
        bias_h[:, 4 + L * 4:8 + L * 4] = bg_all[L].reshape(KCH, 128).T
    shared["biases"] = np.ascontiguousarray(bias_h)

    counts = np.zeros((B, W), np.int64)
    for b in range(B):
        counts[b] = np.bincount(seg[b], minlength=W)[:W]
    assert (counts >= 1).all(), "empty segments unsupported"
    ntaps = max(int(counts.max()), 2)
    starts = np.zeros((B, W), np.int64)
    starts[:, 1:] = np.cumsum(counts, axis=1)[:, :-1]
    ends = starts + counts - 1

    in_maps = []
    gmax = np.zeros(8, np.int64)
    for b in range(B):
        cidx = bt[b] + VOCAB * (bpe[b] + 2 * wrd[b])
        gl = np.empty(8 * ntaps * 128, np.int64)
        for wc in range(8):
            nvec = np.arange(ntaps * 128)
            wv = wc * 128 + (nvec % 128)
            jv = nvec // 128
            sl = slice(wc * ntaps * 128, (wc + 1) * ntaps * 128)
            gl[sl] = np.minimum(starts[b, wv] + jv, ends[b, wv])
            gmax[wc] = max(gmax[wc], int(gl[sl].max()))
        m = dict(shared)
        m["tok_idx"] = _pack_idx(cidx)
        m["gidx"] = np.concatenate(
            [_pack_idx(gl[wc * ntaps * 128:(wc + 1) * ntaps * 128])
             for wc in range(8)], axis=1).copy()
        in_maps.append(m)
    rmaxs = tuple(int(min((g + 128) // 128 * 128, T)) for g in gmax)
    return (ntaps, rmaxs), in_maps


def _run(inputs, trace=False, **kwargs):
    key, in_maps = prepare(inputs)
    nc = _program(key)
    res = run_bass_kernel_spmd(
        nc, in_maps, core_ids=list(range(NCORES)), trace=trace, **kwargs)
    out = np.stack([res.results[b]["out"] for b in range(B)], axis=0)
    return out.astype(np.float32), res


def kernel(**inputs) -> np.ndarray:
    out, _ = _run(inputs, trace=False)
    return out


def run_traced(inputs, **kwargs):
    return _run(inputs, trace=True, **kwargs)
